# revision 15
# baseline (speedup 1.0000x reference)
"""Multi-head attention Trainium2 kernel.

B=4, S=1024, D=1024, H=16, hd=64, f32 reference. 8 NeuronCores:
core c handles batch b=c//2, head-group g=c%2 (8 heads each) —
tensor-parallel over heads within a batch; the host sums the two
partial output projections per batch (the "all-reduce" of the
sharding hint) and adds bo.

All matmul operands are staged bf16 on the host (x, Wq/Wk/Wv, Wo, the
0/1 mask, a pre-broadcast V-bias panel); psum accumulation stays f32.
The 1/sqrt(hd) scale is folded into Wq/bq on the host.

Device dataflow (per core), everything feature-major so there are no
on-device transposes:
  qT[c,s] = sum_i Wq[i,c] xT[i,s] + bq          (lhsT=Wq tile, rhs=xT)
  kT      = k_raw + bk
  V[s,c]  = sum_i xT[i,s] Wv[i,c] + bv          (token-major)
  ST[k,q] = kT.T @ qT          (scores transposed; per k-tile the two
            heads of a pair go to separate 2-bank psum tiles SA/SB via
            tile_position row packing, so each gets ONE [128,1024] exp)
  PT      = exp(ST)            (single ACT instr per head per k-tile,
            psum->sbuf bf16; softmax max-subtraction unnecessary:
            scaled scores are ~N(0,1))
  PT     *= maskT              (one 2x-mode DVE bf16 mult [128,1024])
  valsT[hd,q] = sum over k-tiles V.T @ PT   (two heads col-packed via
            tile_position=(0,0)/(0,64) into ONE psum bank per q-half)
  den[q]  = sum over k-tiles ones[128,1].T @ PT  (four M=1 matmuls
            col-packed at partitions 0/32/64/96 of one den bank)
  vals    = valsT * (1/den)
  out_partial[q,n] = vals.T @ Wo_rows  (bf16 partials; host upcasts)

PSUM budget (8 banks): SA 2 + SB 2 + vpsf 2 + den 1 + weave 1.

Scheduling: the PV/den matmul stream runs LAG k-tiles behind the
scores/exp/mask stream; the reciprocal/broadcast/normalize chain of
pair p is woven into pair p+1's k-loop; rep r+1's QKV projection
chunks (single dedicated weave bank) fill PE bubbles in rep r's
exp-paced attention phase; the [1,S]->[64,S] denominator broadcast
runs on the otherwise idle GPSIMD engine. Per-rep tiles are
double-buffered so consecutive reps pipeline.
"""

import numpy as np

import concourse.bacc as bacc
import concourse.mybir as mybir
import concourse.tile as tile
from concourse import bass_utils
from concourse.alu_op_type import AluOpType

F32 = mybir.dt.float32
BF16 = mybir.dt.bfloat16
AF = mybir.ActivationFunctionType

B, S, D, H, HD = 4, 1024, 1024, 16, 64
NCORES = 8
HPC = 8            # heads per core
VW = HPC * HD      # 512


def build_kernel(debug=False, krep=1, **_ignored):
    nc = bacc.Bacc(trn_type="TRN2", target_bir_lowering=False, debug=False,
                   num_devices=NCORES)

    xT = nc.dram_tensor("xT", [D, S], BF16, kind="ExternalInput").ap()
    maskT = nc.dram_tensor("maskT", [S, S], BF16, kind="ExternalInput").ap()
    wq = nc.dram_tensor("wq", [D, 512], BF16, kind="ExternalInput").ap()
    wk = nc.dram_tensor("wk", [D, 512], BF16, kind="ExternalInput").ap()
    wv = nc.dram_tensor("wv", [D, VW], BF16, kind="ExternalInput").ap()
    bq = nc.dram_tensor("bq", [512], F32, kind="ExternalInput").ap()
    bk = nc.dram_tensor("bk", [512], F32, kind="ExternalInput").ap()
    bvb_d = nc.dram_tensor("bvb", [128, VW], F32, kind="ExternalInput").ap()
    ones_d = nc.dram_tensor("ones1", [128, 1], BF16, kind="ExternalInput").ap()
    wo = nc.dram_tensor("wo", [512, S], BF16, kind="ExternalInput").ap()
    out = nc.dram_tensor("out", [S, S], BF16, kind="ExternalOutput").ap()
    if debug:
        d_q = nc.dram_tensor("d_q", [512, S], F32, kind="ExternalOutput").ap()
        d_k = nc.dram_tensor("d_k", [512, S], F32, kind="ExternalOutput").ap()
        d_vals = nc.dram_tensor("d_vals", [512, S], F32, kind="ExternalOutput").ap()
        d_pt = nc.dram_tensor("d_pt", [256, S], BF16, kind="ExternalOutput").ap()
        d_vraw = nc.dram_tensor("d_vraw", [256, 512], BF16, kind="ExternalOutput").ap()

    with tile.TileContext(nc) as tc:
        with (
            tc.tile_pool(name="persist", bufs=1) as P,
            tc.tile_pool(name="ring", bufs=2) as R,
            tc.tile_pool(name="psum", bufs=1, space="PSUM") as PP,
        ):
            # ---- persistent constants ----
            bq_t = P.tile([128, 4], F32, tag="bq", name="bq")
            bk_t = P.tile([128, 4], F32, tag="bk", name="bk")
            bvb = P.tile([128, VW], F32, tag="bvb", name="bvb")
            ones1 = P.tile([128, 1], BF16, tag="ones1", name="ones1")
            nc.sync.dma_start(bq_t[:], bq.rearrange("(t p) -> p t", p=128))
            nc.sync.dma_start(bk_t[:], bk.rearrange("(t p) -> p t", p=128))
            nc.sync.dma_start(bvb[:], bvb_d)
            nc.sync.dma_start(ones1[:], ones_d)

            def alloc_rep():
                # one monolithic tile per input (single-DMA each);
                # per-chunk views keep the rest of the code unchanged
                t = {}
                big = {}
                for key, cols, n, bufs in (("mN", S, 8, 1), ("xT", S, 8, 2),
                                           ("wq", 512, 8, 2),
                                           ("wk", 512, 8, 2),
                                           ("wv", VW, 8, 2),
                                           ("wo", S, 4, 1)):
                    big[key] = R.tile([128, n * cols], BF16, tag=f"{key}B",
                                      name=f"{key}B", bufs=bufs)
                    t[key] = [big[key][:, i * cols:(i + 1) * cols]
                              for i in range(n)]
                t["_big"] = big
                t["qT"] = [R.tile([128, S], BF16, tag=f"qT{i}",
                                  name=f"qT{i}") for i in range(4)]
                t["kT"] = [R.tile([128, S], BF16, tag=f"kT{i}",
                                  name=f"kT{i}") for i in range(4)]
                t["vA"] = [R.tile([128, VW], BF16, tag=f"vA{i}",
                                  name=f"vA{i}") for i in range(8)]
                t["vals"] = [R.tile([128, S], BF16, tag=f"vals{i}",
                                    name=f"vals{i}", bufs=1)
                             for i in range(4)]
                return t

            def emit_dmas(t):
                # xT+wv first: the first woven V-projection chunks need
                # them; masks last (only needed by that rep's attention)
                big = t["_big"]
                for key, src, cols, n in (("xT", xT, S, 8), ("wv", wv, VW, 8),
                                          ("wq", wq, 512, 8),
                                          ("wk", wk, 512, 8),
                                          ("wo", wo, S, 4),
                                          ("mN", maskT, S, 8)):
                    nc.sync.dma_start(
                        big[key].rearrange("p (a s) -> p a s", a=n),
                        src.rearrange("(a p) s -> p a s", p=128))

            def stage1_chunks(t, tag_iter):
                """QKV projection as 24 independent closures, each one
                psum-accumulation group on a single bank, so they can be
                woven into the previous rep's exp-paced attention loop to
                fill PE bubbles."""
                chunks = []

                def v_group(st):
                    def go():
                        ts_ = slice(st * 128, (st + 1) * 128)
                        pv = PP.tile([128, VW], F32, tag=next(tag_iter),
                                     name="pv")
                        with nc.allow_low_precision(reason="matmul feed"):
                            for i in range(8):
                                nc.tensor.matmul(pv[:], t["xT"][i][:, ts_],
                                                 t["wv"][i][:],
                                                 start=(i == 0),
                                                 stop=(i == 7))
                            nc.vector.tensor_tensor(t["vA"][st][:],
                                                    pv[:], bvb[:],
                                                    AluOpType.add)
                    return go

                def qk_group(which, tt, sh):
                    def go():
                        cs = slice(tt * 128, (tt + 1) * 128)
                        ss = slice(sh * 512, (sh + 1) * 512)
                        w = t["wq"] if which == "q" else t["wk"]
                        dst = t["qT"] if which == "q" else t["kT"]
                        bias = bq_t if which == "q" else bk_t
                        pq = PP.tile([128, 512], F32, tag=next(tag_iter),
                                     name="pq")
                        with nc.allow_low_precision(reason="matmul feed"):
                            for i in range(8):
                                nc.tensor.matmul(pq[:], w[i][:, cs],
                                                 t["xT"][i][:, ss],
                                                 start=(i == 0),
                                                 stop=(i == 7))
                            nc.vector.tensor_scalar(dst[tt][:, ss], pq[:],
                                                    bias[:, tt:tt + 1],
                                                    None, AluOpType.add)
                    return go

                for st in range(8):
                    chunks.append(v_group(st))
                for tt in range(4):
                    for sh in range(2):
                        chunks.append(qk_group("q", tt, sh))
                        chunks.append(qk_group("k", tt, sh))
                return chunks

            def tag_weave():
                while True:
                    yield "WV"

            def tag_prologue():
                tags = ["vp0", "vp1", "DEN", "WV"]
                i = 0
                while True:
                    yield tags[i % 4]
                    i += 1

            # ---- software-pipelined rep loop: rep r's attention weaves
            # in rep r+1's QKV projection chunks (dedicated weave bank) so
            # the exp-paced attention phase keeps the PE busy ----
            rec = [R.tile([1, S], F32, tag=f"rec{hh}", name=f"rec{hh}",
                          bufs=1) for hh in range(2)]
            dstg = [R.tile([1, S], F32, tag=f"dstg{hh}", name=f"dstg{hh}",
                           bufs=1) for hh in range(2)]

            def norm_chunks(t, p, vraw, bcs_l):
                """Pair p's normalization, split into 4 closures woven
                into the next pair's k-loop: per hh, GPSIMD broadcasts of
                the reciprocal dens into [64,512] tiles, then per hh two
                [64,512] multiplies.  (partition_broadcast cannot write
                at a 64-partition offset, so the bcs tiles stay base-0
                and the multiply shifts partitions.)"""
                def chain(hh):
                    for qh in range(2):
                        qs = slice(qh * 512, (qh + 1) * 512)
                        nc.gpsimd.partition_broadcast(
                            bcs_l[2 * hh + qh][:], rec[hh][0:1, qs])

                def mults(hh):
                    hs = slice(64 * hh, 64 * hh + 64)
                    with nc.allow_low_precision(reason="matmul feed"):
                        for qh in range(2):
                            qs = slice(qh * 512, (qh + 1) * 512)
                            nc.vector.tensor_tensor(
                                t["vals"][p][hs, qs],
                                vraw[2 * hh + qh][:],
                                bcs_l[2 * hh + qh][:], AluOpType.mult)

                return [lambda: chain(0), lambda: mults(0),
                        lambda: chain(1), lambda: mults(1)]

            def attention(t, weave):
                """Attention stage for this rep's tiles. `weave` is a list
                of closures (next rep's stage-1 groups) consumed one per
                k-tile slot to fill PE bubbles."""
                pending = []
                qT, kT, vA, mN = t["qT"], t["kT"], t["vA"], t["mN"]
                for p in range(4):   # head pair
                    vpsf = [PP.tile([128, 512], F32, tag=f"vp{qh}",
                                    name=f"vp{qh}") for qh in range(2)]
                    denp = PP.tile([128, 512], F32, tag="DEN", name="denp")
                    LAG = 2          # PV runs LAG k-tiles behind scores
                    pts = {}
                    for kt in range(8 + LAG):
                        if weave and p >= 1:
                            weave.pop(0)()
                        ks = slice(kt * 128, (kt + 1) * 128)
                        if kt < 8:
                            sAB = [PP.tile([128, 1024], F32, tag=tg,
                                           name=tg.lower())
                                   for tg in ("SA", "SB")]
                            for qh in range(2):
                                qs = slice(qh * 512, (qh + 1) * 512)
                                for hh in range(2):
                                    ds = slice(hh * 64, (hh + 1) * 64)
                                    nc.tensor.matmul(
                                        sAB[hh][:, qs], kT[p][ds, ks],
                                        qT[p][ds, qs],
                                        start=True, stop=True,
                                        tile_position=(hh * 64, 0))
                            ptab = []
                            for hh in range(2):
                                pt = R.tile([128, 1024], BF16,
                                            tag=f"pt{hh}", name=f"pt{hh}",
                                            bufs=3)
                                nc.scalar.activation(pt[:], sAB[hh][:],
                                                     AF.Exp)
                                with nc.allow_low_precision(
                                        reason="matmul feed"):
                                    nc.vector.tensor_tensor(
                                        pt[:], pt[:], mN[kt][:],
                                        AluOpType.mult)
                                ptab.append(pt)
                            pts[kt] = ptab
                            if debug and p == 0 and kt == 0:
                                for hh in range(2):
                                    nc.sync.dma_start(
                                        d_pt[hh * 128:(hh + 1) * 128, :],
                                        ptab[hh][:])
                        if kt >= LAG:
                            ptab = pts.pop(kt - LAG)
                            first, last = kt == LAG, kt == 7 + LAG
                            for qh in range(2):
                                qs = slice(qh * 512, (qh + 1) * 512)
                                for hh in range(2):
                                    h = 2 * p + hh
                                    nc.tensor.matmul(
                                        vpsf[qh][64 * hh:64 * hh + 64, :],
                                        vA[kt - LAG][:,
                                                     h * HD:(h + 1) * HD],
                                        ptab[hh][:, qs],
                                        start=first, stop=last,
                                        tile_position=(0, 64 * hh))
                            for hh in range(2):
                                for qh in range(2):
                                    qs = slice(qh * 512, (qh + 1) * 512)
                                    c = 64 * hh + 32 * qh
                                    nc.tensor.matmul(
                                        denp[c:c + 1, :], ones1[:],
                                        ptab[hh][:, qs],
                                        start=first, stop=last,
                                        tile_position=(0, c))
                        # weave the previous pair's normalization in
                        if kt in (2, 4, 6, 8) and pending:
                            pending.pop(0)()

                    # Copy-first: dump each vpsf psum tile to SBUF bf16
                    # immediately so the banks free up for the next pair;
                    # copies split ACT/DVE to halve the engine-FIFO
                    # insertion delay.
                    vraw = [R.tile([64, 512], BF16, tag="vraw",
                                   name="vraw", bufs=5)
                            for _ in range(4)]
                    with nc.allow_low_precision(reason="matmul feed"):
                        for hh in range(2):
                            hs = slice(64 * hh, 64 * hh + 64)
                            nc.scalar.activation(vraw[2 * hh][:],
                                                 vpsf[0][hs, :],
                                                 AF.Identity)
                            nc.vector.tensor_copy(vraw[2 * hh + 1][:],
                                                  vpsf[1][hs, :])
                    # free the DEN bank immediately: stage den rows to
                    # SBUF right away (ACT for hh0, DVE for hh1), then
                    # reciprocals from SBUF
                    for hh in range(2):
                        for qh in range(2):
                            qs = slice(qh * 512, (qh + 1) * 512)
                            c = 64 * hh + 32 * qh
                            if hh == 0:
                                nc.scalar.activation(dstg[0][0:1, qs],
                                                     denp[c:c + 1, :],
                                                     AF.Identity)
                            else:
                                nc.vector.tensor_copy(dstg[1][0:1, qs],
                                                      denp[c:c + 1, :])
                    for hh in range(2):
                        nc.vector.reciprocal_approx_fast(rec[hh][:],
                                                         dstg[hh][:])
                    bcs_l = [R.tile([64, 512], F32, tag="bcs",
                                    name="bcs", bufs=4) for _ in range(4)]
                    if debug and p == 0:
                        for i in range(4):
                            nc.sync.dma_start(
                                d_vraw[i * 64:(i + 1) * 64, :],
                                vraw[i][:])
                    if p == 3:
                        # Last pair: no next k-loop to weave into; run
                        # the broadcast/normalize chains back to back.
                        for hh in range(2):
                            hs = slice(64 * hh, 64 * hh + 64)
                            for qh in range(2):
                                qs = slice(qh * 512, (qh + 1) * 512)
                                nc.gpsimd.partition_broadcast(
                                    bcs_l[2 * hh + qh][:],
                                    rec[hh][0:1, qs])
                            with nc.allow_low_precision(
                                    reason="matmul feed"):
                                for qh in range(2):
                                    qs = slice(qh * 512, (qh + 1) * 512)
                                    nc.vector.tensor_tensor(
                                        t["vals"][p][hs, qs],
                                        vraw[2 * hh + qh][:],
                                        bcs_l[2 * hh + qh][:],
                                        AluOpType.mult)
                    else:
                        pending.extend(norm_chunks(t, p, vraw, bcs_l))

            def outproj(t, weave):
                """Output projection in 2 waves of 4 qt over the psum
                banks (free once pair 3's accumulators are copied out);
                within each wave the pi<3 matmuls come first so head-pair
                3's normalization tail is covered. Leftover weave chunks
                are drained up front, adding further PE cover."""
                while weave:
                    weave.pop(0)()
                vals, wo_t = t["vals"], t["wo"]

                def wave_slots():
                    sa = PP.tile([128, 1024], F32, tag="SA", name="sa")
                    sb = PP.tile([128, 1024], F32, tag="SB", name="sb")
                    slots = [sa[:, 0:512], sa[:, 512:1024],
                             sb[:, 0:512], sb[:, 512:1024]]
                    for tg in ("vp0", "vp1", "DEN", "WV"):
                        slots.append(PP.tile([128, 512], F32, tag=tg,
                                             name="po")[:])
                    return slots

                for wave in range(2):
                    po_w = {}
                    slots = wave_slots()
                    for qt in range(wave * 4, wave * 4 + 4):
                        qs = slice(qt * 128, (qt + 1) * 128)
                        for nh in range(2):
                            ns = slice(nh * 512, (nh + 1) * 512)
                            po = slots[2 * (qt - wave * 4) + nh]
                            po_w[(qt, nh)] = po
                            for pi in range(3):
                                nc.tensor.matmul(po, vals[pi][:, qs],
                                                 wo_t[pi][:, ns],
                                                 start=(pi == 0), stop=False)
                    for qt in range(wave * 4, wave * 4 + 4):
                        qs = slice(qt * 128, (qt + 1) * 128)
                        ot = R.tile([128, S], BF16, tag="ot", name="ot",
                                    bufs=2)
                        for nh in range(2):
                            ns = slice(nh * 512, (nh + 1) * 512)
                            po = po_w[(qt, nh)]
                            nc.tensor.matmul(po, vals[3][:, qs],
                                             wo_t[3][:, ns],
                                             start=False, stop=True)
                            with nc.allow_low_precision(
                                    reason="matmul feed"):
                                nc.scalar.activation(ot[:, ns], po,
                                                     AF.Identity)
                        nc.sync.dma_start(out[qs, :], ot[:])

            # prologue: rep 0's tiles + full stage 1 inline
            cur = alloc_rep()
            emit_dmas(cur)
            for c in stage1_chunks(cur, tag_prologue()):
                c()
            for rep in range(krep):
                if rep + 1 < krep:
                    nxt = alloc_rep()
                    emit_dmas(nxt)
                    weave = stage1_chunks(nxt, tag_weave())
                else:
                    nxt, weave = None, []
                attention(cur, weave)
                if debug:
                    for pi in range(4):
                        nc.sync.dma_start(
                            d_vals[pi * 128:(pi + 1) * 128, 0:512],
                            cur["vals"][pi][:].bitcast(F32))
                    for tt in range(4):
                        nc.sync.dma_start(d_q[tt * 128:(tt + 1) * 128, 0:512],
                                          cur["qT"][tt][:].bitcast(F32))
                        nc.sync.dma_start(d_k[tt * 128:(tt + 1) * 128, 0:512],
                                          cur["kT"][tt][:].bitcast(F32))
                outproj(cur, weave)
                cur = nxt

    nc.compile()
    return nc


_NC_CACHE = {}


def _get_nc():
    if "nc" not in _NC_CACHE:
        _NC_CACHE["nc"] = build_kernel()
    return _NC_CACHE["nc"]


def _bf16(a):
    import jax.numpy as jnp
    return np.asarray(jnp.asarray(a, dtype=jnp.bfloat16))


def shard_inputs(x, mask, Wqkv, bqkv, Wo, bo):
    """Per-core input dicts. Layout/slicing/dtype staging only."""
    x = np.asarray(x, dtype=np.float32)
    mask = np.asarray(mask, dtype=np.int32)
    Wqkv = np.asarray(Wqkv, dtype=np.float32)
    bqkv = np.asarray(bqkv, dtype=np.float32)
    Wo = np.asarray(Wo, dtype=np.float32)

    scale = 1.0 / np.sqrt(HD)
    Wr = Wqkv.reshape(D, H, 3, HD)
    br = bqkv.reshape(H, 3, HD)
    ones1 = np.ones((128, 1), dtype=np.float32)
    in_maps = []
    for c in range(NCORES):
        b, g = c // 2, c % 2
        hs = slice(g * HPC, (g + 1) * HPC)
        bvb = np.tile(br[hs, 2, :].reshape(1, VW), (128, 1))
        in_maps.append({
            "xT": _bf16(x[b].T),
            "maskT": _bf16(mask[b].T),
            "wq": _bf16(Wr[:, hs, 0, :].reshape(D, 512) * scale),
            "wk": _bf16(Wr[:, hs, 1, :].reshape(D, 512)),
            "wv": _bf16(Wr[:, hs, 2, :].reshape(D, VW)),
            "bq": np.ascontiguousarray(br[hs, 0, :].reshape(512) * scale),
            "bk": np.ascontiguousarray(br[hs, 1, :].reshape(512)),
            "bvb": np.ascontiguousarray(bvb),
            "ones1": _bf16(ones1),
            "wo": _bf16(Wo[g * 512:(g + 1) * 512, :]),
        })
    return in_maps


def combine_outputs(results, bo):
    bo = np.asarray(bo, dtype=np.float32)
    out = np.empty((B, S, D), dtype=np.float32)
    for b in range(B):
        out[b] = (np.asarray(results[2 * b]["out"], dtype=np.float32)
                  + np.asarray(results[2 * b + 1]["out"], dtype=np.float32)
                  + bo)
    return out


def kernel(x, mask, Wqkv, bqkv, Wo, bo):
    nc = _get_nc()
    in_maps = shard_inputs(x, mask, Wqkv, bqkv, Wo, bo)
    res = bass_utils.run_bass_kernel_spmd(nc, in_maps,
                                          core_ids=list(range(NCORES)))
    return combine_outputs(res.results, bo)
